# revision 1
# baseline (speedup 1.0000x reference)
"""Trainium2 Bass kernel for fused GEMM + row-LayerNorm + row-Softmax.

Computes, for x [M=65536, K=1024], weight [N=512, K], bias/gamma/beta [N],
scale [1]:
    acc  = x @ weight.T + bias
    norm = (acc - mean_row) / sqrt(var_row + EPS)
    out  = softmax(norm * gamma + beta) * scale, axis=-1)   (row softmax)

Sharding: data-parallel along M across 8 NeuronCores; weight/bias/gamma/
beta/scale replicated.  Host-side prep transposes x to [K, M] so each
k-chunk loads directly as the matmul's stationary operand (lhsT).
"""

import os

import numpy as np

import concourse.bass as bass
import concourse.tile as tile
from concourse import bacc, mybir
from concourse.bass_utils import run_bass_kernel_spmd

EPS = 1e-5
K = 1024
N = 512
M = 65536
N_CORES = 8
M_CORE = M // N_CORES
P = 128
KC = K // P  # k-chunks of 128
# 512-col x blocks + groups of 2 subtiles pipeline best (cost model:
# 153.5us vs 162 at 1024/4): shorter per-block DMA prologue, deeper
# 2+2+2+2 PSUM rotation; 1KB DMA lines keep descriptor overhead sane.
MSPAN = int(os.environ.get("BASS_MSPAN", "512"))  # m-cols per x block

F32 = mybir.dt.float32
F32R = mybir.dt.float32r

# matmul input mode: "f32r" (fp32 storage, full-rate reduced-precision PE
# math), "f32" (exact, 4x slower PE), "f16"/"bf16" (half storage+DMA).
# f16 measured ~15% faster end-to-end (x DMA halves) at rel-err 3.4e-4
# vs f32r's 1.6e-4 — both far inside tolerance.
MM_MODE = os.environ.get("BASS_MM_MODE", "f16")
# rstd path: "newton" = DVE-only rsqrt (fast-inverse-sqrt seed + 2 Newton
# steps) so Exp is the only ACT LUT function and the table loads exactly
# once (each swap costs ~1.3us x 32); "lnexp" = ACT ln+exp fallback.
# "pow" is rejected by walrus ISA.
RSTD_MODE = os.environ.get("BASS_RSTD", "newton")
HB = int(os.environ.get("BASS_HB", "1"))  # subtiles per stats/exp group
OBLK_BUFS = int(os.environ.get("BASS_OBLK", "2"))
EOUT_BUFS = int(os.environ.get("BASS_EOUT", "3"))
# issue output DMAs from this engine's HWDGE ring ("sync" or "scalar") —
# scalar decouples store FIFO ordering from the x-load ring on SP
OUT_ENG = os.environ.get("BASS_OUT_ENG", "scalar")
XBUFS = int(os.environ.get("BASS_XBUFS", "3"))

_NC_CACHE: dict = {}


def _mm_dt(mode):
    return {
        "f32r": F32R,  # f32 bytes, reduced-precision full-rate PE math
        "f32": F32,
        "f16": mybir.dt.float16,
        "bf16": mybir.dt.bfloat16,
    }[mode]


def _np_dt(mode):
    import ml_dtypes

    return {
        "f32r": np.float32,
        "f32": np.float32,
        "f16": np.float16,
        "bf16": ml_dtypes.bfloat16,
    }[mode]


def _build(m_core, mode, fast, gs_const, repeat=1):
    """Build + compile the per-core Bass module.

    fast: gamma*scale and beta*scale are constant across N -> fold the
    constant gamma*scale into rstd and drop the beta shift (softmax is
    invariant to per-row constants).  gs_const is that constant.

    repeat: run the whole pass `repeat` times back-to-back (same I/O) —
    used by the timing harness to measure the marginal cost of one pass.
    """
    x_dt = _mm_dt(mode)
    nc = bacc.Bacc(
        "TRN2", target_bir_lowering=False, debug=False, num_devices=N_CORES
    )

    xt = nc.dram_tensor("xt", [K, m_core], x_dt, kind="ExternalInput").ap()
    wt = nc.dram_tensor("wt", [K, N], x_dt, kind="ExternalInput").ap()
    # bias enters the GEMM via a rank-1 matmul; f32r streams at full rate
    # for N>=256 and keeps full input bytes, independent of the x/w mode.
    b_dt = F32R
    bias_d = nc.dram_tensor("bias", [N], b_dt, kind="ExternalInput").ap()
    ones_d = nc.dram_tensor("ones", [P], b_dt, kind="ExternalInput").ap()
    if not fast:
        g2_d = nc.dram_tensor("gamma2", [N], F32, kind="ExternalInput").ap()
        b2_d = nc.dram_tensor("beta2", [N], F32, kind="ExternalInput").ap()
    out = nc.dram_tensor("out", [m_core, N], F32, kind="ExternalOutput").ap()

    xt_r = xt.rearrange("(c p) m -> c p m", p=P)
    wt_r = wt.rearrange("(c p) n -> c p n", p=P)

    def bcast(ap_1d, parts):
        return bass.AP(
            tensor=ap_1d.tensor, offset=ap_1d.offset, ap=[[0, parts]] + list(ap_1d.ap)
        )

    n_mblk = m_core // MSPAN
    n_sub = MSPAN // P

    with tile.TileContext(nc) as tc:
        with (
            tc.tile_pool(name="singles", bufs=1) as singles,
            tc.tile_pool(name="xin", bufs=XBUFS) as xin,
            tc.tile_pool(name="psum", bufs=8, space="PSUM") as ppool,
            tc.tile_pool(name="stats", bufs=8) as stats_p,
            tc.tile_pool(name="eout", bufs=EOUT_BUFS) as eout_p,
            tc.tile_pool(name="oblk", bufs=OBLK_BUFS) as oblk_p,
        ):
            # --- one-time setup ---
            wt_sb = singles.tile([P, KC, N], x_dt)
            for c in range(KC):
                nc.sync.dma_start(out=wt_sb[:, c, :], in_=wt_r[c])
            ones_sb = singles.tile([1, P], b_dt)
            nc.sync.dma_start(out=ones_sb, in_=ones_d.rearrange("(a p) -> a p", a=1))
            bias_sb = singles.tile([1, N], b_dt)
            nc.sync.dma_start(out=bias_sb, in_=bias_d.rearrange("(a n) -> a n", a=1))
            eps_sb = singles.tile([P, 1], F32)
            nc.vector.memset(eps_sb, EPS)
            if not fast:
                g2b = singles.tile([P, N], F32)
                nc.sync.dma_start(out=g2b, in_=bcast(g2_d, P))
                b2b = singles.tile([P, N], F32)
                nc.sync.dma_start(out=b2b, in_=bcast(b2_d, P))

            gs = None if fast and gs_const == 1.0 else float(gs_const)
            out_r = out.rearrange("(b j p) n -> b j p n", j=n_sub, p=P)

            # --- main loop ---
            for rep in range(repeat):
                for ib in range(n_mblk):
                    ms = ib * MSPAN
                    x_tile = xin.tile([P, KC, MSPAN], x_dt, tag="x")
                    if rep == 0 and ib == 0:
                        # prologue: per-chunk DMAs so the first matmul only
                        # waits on chunk 0 (~64KB), not the whole tile
                        for c in range(KC):
                            nc.sync.dma_start(
                                out=x_tile[:, c, :],
                                in_=xt_r[c, :, ms : ms + MSPAN],
                            )
                    else:
                        nc.sync.dma_start(
                            out=x_tile,
                            in_=xt_r[:, :, ms : ms + MSPAN].rearrange(
                                "c p m -> p c m"
                            ),
                        )
                    o_blk = oblk_p.tile([P, n_sub, N], F32, tag="oblk")
                    for g in range(n_sub // HB):
                        # GEMM + stats for HB subtiles; their PSUM accs stay
                        # live so the batched rstd feeds the exps.
                        accs = []
                        mvb = stats_p.tile([P, HB, 2], F32, tag="mv")
                        for h in range(HB):
                            j = g * HB + h
                            acc = ppool.tile([P, N], F32, space="PSUM", tag="acc")
                            accs.append(acc)
                            for c in range(KC):
                                nc.tensor.matmul(
                                    acc,
                                    x_tile[:, c, j * P : (j + 1) * P],
                                    wt_sb[:, c, :],
                                    start=(c == 0),
                                    stop=False,
                                )
                            # rank-1 broadcast add of bias: ones.T @ bias.
                            # (A DVE in-place PSUM add was tried instead and
                            # predicts 6us WORSE overall: it serializes
                            # between the GEMM and bn_stats, lengthening the
                            # critical path, while the PE absorbs the rank-1
                            # stream inside its pipelined matmul flow.)
                            nc.tensor.matmul(
                                acc, ones_sb, bias_sb, start=False, stop=True
                            )
                            st = stats_p.tile([P, 6], F32, tag="st")
                            nc.vector.bn_stats(out=st, in_=acc)
                            nc.vector.bn_aggr(out=mvb[:, h, :], in_=st)

                        # Batched rstd = (var+eps)^-0.5 for the HB subtiles.
                        rstdb = stats_p.tile([P, HB], F32, tag="rstdb")
                        if RSTD_MODE == "newton":
                            # DVE-only rsqrt: fast-inverse-sqrt seed (float
                            # bits as a number: K - bits/2) + 2 Newton steps.
                            # Keeps Exp as the ONLY ACT function -> the ACT
                            # LUT loads once for the whole kernel.
                            I32 = mybir.dt.int32
                            xe = stats_p.tile([P, HB], F32, tag="xe")
                            nc.vector.tensor_scalar_add(
                                out=xe, in0=mvb[:, :, 1], scalar1=EPS
                            )
                            bi = stats_p.tile([P, HB], F32, tag="bi")
                            nc.vector.tensor_copy(bi, xe.bitcast(I32))
                            y0f = stats_p.tile([P, HB], F32, tag="y0f")
                            nc.vector.tensor_scalar(
                                out=y0f,
                                in0=bi,
                                scalar1=-0.5,
                                scalar2=float(0x5F3759DF),
                                op0=mybir.AluOpType.mult,
                                op1=mybir.AluOpType.add,
                            )
                            y0i = stats_p.tile([P, HB], I32, tag="y0i")
                            nc.vector.tensor_copy(y0i, y0f)
                            y = y0i.bitcast(F32)
                            for it in range(2):
                                t = stats_p.tile([P, HB], F32, tag=f"nt{it}")
                                nc.vector.tensor_mul(t, xe, y)
                                nc.vector.tensor_mul(t, t, y)
                                nc.vector.tensor_scalar(
                                    out=t,
                                    in0=t,
                                    scalar1=-0.5,
                                    scalar2=1.5,
                                    op0=mybir.AluOpType.mult,
                                    op1=mybir.AluOpType.add,
                                )
                                dst = rstdb if it == 1 else stats_p.tile(
                                    [P, HB], F32, tag=f"ny{it}"
                                )
                                nc.vector.tensor_mul(dst, t, y)
                                y = dst
                        elif RSTD_MODE == "pow":
                            nc.vector.tensor_scalar(
                                out=rstdb,
                                in0=mvb[:, :, 1],
                                scalar1=EPS,
                                scalar2=-0.5,
                                op0=mybir.AluOpType.add,
                                op1=mybir.AluOpType.pow,
                            )
                        else:
                            # exp(-0.5*ln(var+eps)); Ln/Exp batched so the
                            # ACT LUT swaps twice per group, not per subtile
                            lnb = stats_p.tile([P, HB], F32, tag="lnb")
                            nc.scalar.activation(
                                out=lnb,
                                in_=mvb[:, :, 1],
                                func=mybir.ActivationFunctionType.Ln,
                                bias=eps_sb,
                            )
                            nc.scalar.activation(
                                out=rstdb,
                                in_=lnb,
                                func=mybir.ActivationFunctionType.Exp,
                                scale=-0.5,
                            )
                        if fast and gs is not None:
                            nc.vector.tensor_scalar_mul(
                                out=rstdb, in0=rstdb, scalar1=gs
                            )
                        # nmrb = -mean * rstd (per subtile column)
                        nmrb = stats_p.tile([P, HB], F32, tag="nmrb")
                        nc.vector.scalar_tensor_tensor(
                            out=nmrb,
                            in0=mvb[:, :, 0],
                            scalar=-1.0,
                            in1=rstdb,
                            op0=mybir.AluOpType.mult,
                            op1=mybir.AluOpType.mult,
                        )

                        for h in range(HB):
                            j = g * HB + h
                            acc = accs[h]
                            e_t = eout_p.tile([P, N], F32, tag="e")
                            ssum = stats_p.tile([P, 1], F32, tag="ssum")
                            if fast:
                                # e = exp(acc*rstd - mean*rstd)
                                nc.scalar.activation(
                                    out=e_t,
                                    in_=acc,
                                    func=mybir.ActivationFunctionType.Exp,
                                    bias=nmrb[:, h : h + 1],
                                    scale=rstdb[:, h : h + 1],
                                    accum_out=ssum,
                                )
                            else:
                                # u = ((acc - mean) * gamma2) * rstd + beta2
                                u1 = eout_p.tile([P, N], F32, tag="u1")
                                nc.vector.scalar_tensor_tensor(
                                    out=u1,
                                    in0=acc,
                                    scalar=mvb[:, h, 0:1],
                                    in1=g2b,
                                    op0=mybir.AluOpType.subtract,
                                    op1=mybir.AluOpType.mult,
                                )
                                u = eout_p.tile([P, N], F32, tag="u")
                                nc.vector.scalar_tensor_tensor(
                                    out=u,
                                    in0=u1,
                                    scalar=rstdb[:, h : h + 1],
                                    in1=b2b,
                                    op0=mybir.AluOpType.mult,
                                    op1=mybir.AluOpType.add,
                                )
                                nmax = stats_p.tile([P, 1], F32, tag="nmax")
                                nc.vector.tensor_reduce(
                                    out=nmax,
                                    in_=u,
                                    axis=mybir.AxisListType.X,
                                    op=mybir.AluOpType.max,
                                    negate=True,
                                )
                                nc.scalar.activation(
                                    out=e_t,
                                    in_=u,
                                    func=mybir.ActivationFunctionType.Exp,
                                    bias=nmax,
                                    accum_out=ssum,
                                )

                            rden = stats_p.tile([P, 1], F32, tag="rden")
                            nc.vector.reciprocal(out=rden, in_=ssum)
                            nc.vector.tensor_scalar_mul(
                                out=o_blk[:, j, :], in0=e_t, scalar1=rden
                            )
                    out_eng = nc.scalar if OUT_ENG == "scalar" else nc.sync
                    out_eng.dma_start(
                        out=out_r[ib].rearrange("j p n -> p j n"),
                        in_=o_blk,
                    )

    nc.compile()
    return nc


def _get_nc(m_core, mode, fast, gs_const):
    key = (m_core, mode, fast, gs_const if fast else None)
    if key not in _NC_CACHE:
        _NC_CACHE[key] = _build(m_core, mode, fast, gs_const)
    return _NC_CACHE[key]


def _prep(x, weight, bias, gamma, beta, scale, mode):
    """Host-side prep shared by kernel() and the test harness."""
    np_dt = _np_dt(mode)
    s = float(np.asarray(scale).reshape(-1)[0])
    g2 = (np.asarray(gamma, np.float32) * s).astype(np.float32)
    b2 = (np.asarray(beta, np.float32) * s).astype(np.float32)
    fast = bool(np.all(g2 == g2[0]) and np.all(b2 == b2[0]))
    gs_const = float(g2[0]) if fast else 0.0
    xt = np.ascontiguousarray(np.asarray(x, np.float32).T).astype(np_dt)
    wt = np.ascontiguousarray(np.asarray(weight, np.float32).T).astype(np_dt)
    return xt, wt, np.asarray(bias, np.float32), g2, b2, fast, gs_const


def kernel(x, weight, bias, gamma, beta, scale):
    mode = MM_MODE
    xt, wt, bias_f, g2, b2, fast, gs_const = _prep(
        x, weight, bias, gamma, beta, scale, mode
    )
    m_core = x.shape[0] // N_CORES
    nc = _get_nc(m_core, mode, fast, gs_const)

    in_maps = []
    for c in range(N_CORES):
        im = {
            "xt": np.ascontiguousarray(xt[:, c * m_core : (c + 1) * m_core]),
            "wt": wt,
            "bias": bias_f,
            "ones": np.ones(P, np.float32),
        }
        if not fast:
            im["gamma2"] = g2
            im["beta2"] = b2
        in_maps.append(im)

    res = run_bass_kernel_spmd(nc, in_maps, list(range(N_CORES))).results
    return np.concatenate([res[c]["out"] for c in range(N_CORES)], axis=0)

